# revision 57
# baseline (speedup 1.0000x reference)
"""DeformConv1d Trainium2 Bass kernel.

Problem: x[4,512,4096] f32, offsets[4,1,4090,7] f32, weight[512,512,7], bias[512]
  T[b,o,k]   = clamp(o + k + offsets[b,0,o,k], o, o+6)
  samp[b,c,o,k] = linear-interp of x[b,c,:] at T
  out[b,d,o] = sum_{c,k} samp[b,c,o,k] * weight[d,c,k] + bias[d]

Key identity: the clamp keeps every sample inside [o, o+6], so output o only
touches taps p in [o, o+7], and the interpolation weight of tap p is exactly
relu(1 - |p - T|).  With an o-tile of 121 the tap band is exactly 128 wide:

  out[o,d] = sum_{k, dp<128} S_k[dp, o] * Y[o0+dp, k, d]
    S_k[dp,o] = relu(1 - |cplus_k[o] - dp|),  cplus_k[o] = clamp(k+off,0,6)+j
    Y[p,k,d]  = sum_c x[c, p] * weight[d, c, k]

Both stages are dense bf16 matmuls on the PE array (f32 PSUM accumulate) and
run back-to-back at the 2-col/cycle bf16 stream rate (216 ns per N=512
matmul, ~86% tensor-engine occupancy).  cplus = clamp(k+off,0,6)+j is
computed once on-chip in f32, staged to DRAM as per-tile contiguous rows,
and expanded across all 128 partitions by a 0-stride DMA on the otherwise
idle software DGE queue (deep prefetch via an 8-buffer pool keeps it off the
critical path; tile 0's broadcast rides the hardware queue, wait-hinted
behind the weight stream to avoid head-of-line blocking it); ACT Abs with
per-partition bias -dp plus ACT relu turn it into the bf16 tap weights.
Bias is folded into the k=3 Y eviction (interp weights sum to 1 per k, so
adding bias[d] to one k's Y adds exactly bias[d] to the output) - no bias
matmul.  The output is stored bf16 (error budget allows it) to halve output
DMA traffic; the host upcasts.  PSUM evictions are split across
vector/scalar so no engine queue head-of-line-blocks stage-2.  The bulk x
stream is gated behind early output DMAs via scheduler wait hints so the
16 DMA engines (which round-robin all rung doorbells) give the head
bandwidth to the weights; the last tile's stage-2 runs in two d-halves so
the final eviction + store overlap its own matmuls.
Sharding: 8 cores = 4 batches x 2 halves of out_len (data parallel, no comm).
"""

import os
import sys

import ml_dtypes
import numpy as np

for _p in ("/opt/trn_rl_repo", os.path.expanduser("~/.axon_site/_ro/trn_rl_repo")):
    if os.path.isdir(_p) and _p not in sys.path:
        sys.path.insert(0, _p)

import concourse.mybir as mybir
import concourse.tile as tile
from concourse import bacc
from concourse.bass_utils import run_bass_kernel_spmd

B, CIN, COUT, L, K = 4, 512, 512, 4096, 7
OUT_LEN = 4090
HALF = 2045          # out positions per core (2 halves per batch)
OT = 121             # o-tile size -> tap band = OT + 7 = 128
TILES = 17           # 17 * 121 = 2057 >= 2045
OPAD = TILES * OT    # 2057 padded out positions per core
XW = (TILES - 1) * OT + 128  # 2064: rightmost x column any tile reads
P = 128
NCK = CIN // P       # 4 c-chunks
F32 = mybir.dt.float32
BF16 = mybir.dt.bfloat16

_prog_cache = {}


def _build_program():
    nc = bacc.Bacc("TRN2", target_bir_lowering=False, debug=False)

    xs_d = nc.dram_tensor("xs", [CIN, XW], BF16, kind="ExternalInput")
    wt_d = nc.dram_tensor("wt", [CIN, K, COUT], BF16, kind="ExternalInput")
    offsT_d = nc.dram_tensor("offsT", [K, OPAD], F32, kind="ExternalInput")
    bias_d = nc.dram_tensor("bias2", [1, COUT], BF16, kind="ExternalInput")
    jrow_d = nc.dram_tensor("jrow", [K, OPAD], F32, kind="ExternalInput")
    kcol_d = nc.dram_tensor("kcol", [K, 1], F32, kind="ExternalInput")
    dpcol_d = nc.dram_tensor("dpcol", [P, 1], F32, kind="ExternalInput")
    out_d = nc.dram_tensor("out", [OPAD, COUT], BF16, kind="ExternalOutput")

    with tile.TileContext(nc) as tc:
        with (
            tc.tile_pool(name="const", bufs=1) as cpool,
            tc.tile_pool(name="cdram", bufs=1, space="DRAM") as dpool,
            tc.tile_pool(name="cbt", bufs=8) as cbpool,
            tc.tile_pool(name="stiles", bufs=4) as stpool,
            tc.tile_pool(name="ytiles", bufs=3) as ypool,
            tc.tile_pool(name="otiles", bufs=3) as opool,
            tc.tile_pool(name="psy", bufs=7, space="PSUM") as psy,
            tc.tile_pool(name="pso", bufs=1, space="PSUM") as pso,
        ):
            # ---- small constants / offsets first (ahead of bulk x/W DMA) ----
            # the offsets -> cexp -> relayout -> cb(0) chain gates tile-0's
            # stage-2, so it leads the software queue; the 128-packet bias
            # broadcast rides the hardware queue instead (tiny, not urgent)
            offsT = cpool.tile([K, OPAD], F32)
            nc.gpsimd.dma_start(offsT[:], offsT_d[:])
            jrow = cpool.tile([K, OPAD], F32)
            nc.gpsimd.dma_start(jrow[:], jrow_d[:])
            kcol = cpool.tile([K, 1], F32)
            nc.gpsimd.dma_start(kcol[:], kcol_d[:])
            dpcol = cpool.tile([P, 1], F32)
            nc.gpsimd.dma_start(dpcol[:], dpcol_d[:])
            bias_bc = cpool.tile([P, COUT], BF16)
            with tc.tile_wait_until(0.006):
                nc.sync.dma_start(bias_bc[:], bias_d[:].partition_broadcast(P))

            # cplus[k, o] = clamp(k + off[k, o], 0, 6) + (o % OT), staged to
            # DRAM so a 0-stride DMA can expand it across partitions per tile
            cexp = cpool.tile([K, OPAD], F32)
            nc.vector.tensor_scalar(
                cexp[:], offsT[:], kcol[:], 0.0,
                mybir.AluOpType.add, mybir.AluOpType.max,
            )
            nc.vector.scalar_tensor_tensor(
                cexp[:], cexp[:], 6.0, jrow[:],
                mybir.AluOpType.min, mybir.AluOpType.add,
            )
            # relayout to per-tile contiguous rows [t, koff+j] in one shot:
            # k<4 at koff=121k, k>=4 at 512+121(k-4)
            c_dram2 = dpool.tile([TILES, 1024], F32)
            nc.gpsimd.dma_start(
                c_dram2[:, 0 : 4 * OT].rearrange("t (k j) -> k t j", k=4),
                cexp[0:4, :].rearrange("k (t j) -> k t j", t=TILES),
            )
            nc.gpsimd.dma_start(
                c_dram2[:, 512 : 512 + 3 * OT].rearrange("t (k j) -> k t j", k=3),
                cexp[4:7, :].rearrange("k (t j) -> k t j", t=TILES),
            )

            # ---- bulk inputs, in tile-0 consumption order ----
            xs = cpool.tile([P, NCK, XW], BF16)
            wt = cpool.tile([P, NCK, K, COUT], BF16)
            xs_src = xs_d[:].rearrange("(ci p) t -> p ci t", p=P)
            wt_src = wt_d[:].rearrange("(ci p) k d -> p ci k d", p=P)
            # head-critical: x band + all weights for tiles 0-2, ungated
            nc.sync.dma_start(xs[:, :, 0:130], xs_src[:, :, 0:130])
            for k in range(K):
                nc.sync.dma_start(wt[:, :, k, :], wt_src[:, :, k, :])
            with tc.tile_wait_until(0.0045):
                nc.sync.dma_start(xs[:, :, 130:391], xs_src[:, :, 130:391])
            # remaining x gated behind early output DMAs via scheduler wait
            # hints, so its packets don't round-robin-steal head bandwidth
            # from the weight stream (DMA engines serve all rung doorbells)
            for lo, hi, w in ((391, 775, 0.0075), (775, 1420, 0.015),
                              (1420, XW, 0.028)):
                with tc.tile_wait_until(w):
                    nc.sync.dma_start(xs[:, :, lo:hi], xs_src[:, :, lo:hi])

            reg = slice(0, 512 + 3 * OT)
            for t in range(TILES):
                o0 = t * OT

                # ---- Y[dp, k, d] for band p in [o0, o0+128) ----
                y_sb = ypool.tile([P, K, COUT], BF16, tag="y_sb")
                for k in range(K):
                    yp = psy.tile([P, COUT], F32, tag="yp")
                    for ci in range(NCK):
                        nc.tensor.matmul(
                            yp[:],
                            xs[:, ci, o0 : o0 + P],
                            wt[:, ci, k, :],
                            start=(ci == 0), stop=(ci == NCK - 1),
                        )
                    if k == 3:
                        # interp weights sum to 1 per k: bias rides on k=3
                        nc.vector.tensor_tensor(
                            y_sb[:, k, :], yp[:], bias_bc[:],
                            mybir.AluOpType.add,
                        )
                    elif k in (0, 1, 2, 4):
                        nc.vector.tensor_copy(y_sb[:, k, :], yp[:])
                    else:
                        nc.scalar.copy(y_sb[:, k, :], yp[:])

                # ---- S_k[dp, o] = relu(1 - |cplus_k - dp|) ----
                # 0-stride DMA expands this tile's cplus row across all 128
                # partitions (4KB contiguous per partition), then ACT Abs with
                # per-partition bias -dp + ACT relu produce the bf16 weights.
                cb = cbpool.tile([P, 512 + 3 * OT], F32, tag="cb")
                s_sb = stpool.tile([P, 1024], BF16, tag="s_sb")
                # broadcasts ride the otherwise idle software DMA queue with
                # deep prefetch via cbpool, keeping the hardware queue free
                # for the bulk x/w input stream at the head; tile 0's rides
                # the fast hardware queue so s_sb(0) beats tile-0 stage-2
                if t == 0:
                    with tc.tile_wait_until(0.0045):
                        nc.sync.dma_start(
                            cb[:],
                            c_dram2[t : t + 1, 0 : 512 + 3 * OT].partition_broadcast(P),
                        )
                else:
                    nc.gpsimd.dma_start(
                        cb[:],
                        c_dram2[t : t + 1, 0 : 512 + 3 * OT].partition_broadcast(P),
                    )
                nc.scalar.activation(
                    cb[:], cb[:],
                    mybir.ActivationFunctionType.Abs,
                    bias=dpcol[:],
                )
                nc.scalar.activation(
                    s_sb[:, reg], cb[:],
                    mybir.ActivationFunctionType.Relu,
                    bias=1.0, scale=-1.0,
                )

                # ---- out[o, d] = sum_k S_k^T Y_k (+bias via k=3) ----
                # last tile runs in two d-halves so the first half's
                # eviction + store overlap the second half's matmuls,
                # shortening the serial tail after the final matmul
                op = pso.tile([P, COUT], F32, tag="op")
                o_sb = opool.tile([P, COUT], BF16, tag="o_sb")
                halves = ((0, 256), (256, COUT)) if t == TILES - 1 else ((0, COUT),)
                for dlo, dhi in halves:
                    for k in range(K):
                        koff = k * OT if k < 4 else 512 + (k - 4) * OT
                        nc.tensor.matmul(
                            op[:OT, dlo:dhi],
                            s_sb[:, koff : koff + OT],
                            y_sb[:, k, dlo:dhi],
                            start=(k == 0), stop=(k == K - 1),
                        )
                    # evict [o, d] bf16 and store; host upcasts + transposes
                    if t % 2 == 0:
                        nc.scalar.copy(o_sb[:OT, dlo:dhi], op[:OT, dlo:dhi])
                    else:
                        nc.vector.tensor_copy(o_sb[:OT, dlo:dhi], op[:OT, dlo:dhi])
                    nc.sync.dma_start(
                        out_d[o0 : o0 + OT, dlo:dhi], o_sb[:OT, dlo:dhi]
                    )

    nc.compile()
    return nc


def _install_axon_ntff_hook():
    """Provide antenv.axon_hooks (absent on this image) so that
    run_bass_kernel_spmd(trace=True) can capture NTFF profiles via the
    axon .so's C ABI.  Mirrors trn_agent_boot.trn_boot."""
    import contextlib
    import ctypes
    import types

    try:
        from antenv.axon_hooks import set_axon_ntff_profile_hook  # noqa: F401
        return
    except ImportError:
        pass

    so_path = "/opt/axon/libaxon_pjrt.so"
    if not os.path.exists(so_path):
        return
    lib = ctypes.CDLL(so_path)
    if not hasattr(lib, "axon_start_nrt_profile"):
        return
    lib.axon_start_nrt_profile.argtypes = [
        ctypes.POINTER(ctypes.c_int64), ctypes.c_size_t,
    ]
    lib.axon_start_nrt_profile.restype = ctypes.c_int64
    lib.axon_stop_nrt_profile.argtypes = [ctypes.c_char_p]
    lib.axon_stop_nrt_profile.restype = ctypes.c_int64

    @contextlib.contextmanager
    def _hook(output_dir, device_ids):
        import jax

        jax.devices()
        if device_ids:
            ids = (ctypes.c_int64 * len(device_ids))(*device_ids)
            rc = lib.axon_start_nrt_profile(ids, len(device_ids))
        else:
            rc = lib.axon_start_nrt_profile(None, 0)
        if rc != 0:
            raise RuntimeError(f"axon_start_nrt_profile rc={rc}")
        try:
            yield
        finally:
            n = lib.axon_stop_nrt_profile(str(output_dir).encode())
            print(f"ntff profile: {n} file(s) written to {output_dir}")

    box = {"h": _hook}
    mod = types.ModuleType("antenv.axon_hooks")
    mod.get_axon_ntff_profile_hook = lambda: box["h"]
    mod.set_axon_ntff_profile_hook = lambda h: box.__setitem__("h", h)
    import antenv

    sys.modules["antenv.axon_hooks"] = mod
    antenv.axon_hooks = mod

    # zero-egress env: skip the artifact upload in the trace path
    from concourse import bass_utils as _bu

    _bu.upload_artifacts = lambda d: f"local:{d}"


def _consts():
    kcol = np.arange(K, dtype=np.float32).reshape(K, 1).copy()
    # ACT Abs computes |x + bias|; bias = -dp gives |cplus - dp|
    dpcol = -np.arange(P, dtype=np.float32).reshape(P, 1)
    jrow = np.tile(
        (np.arange(OPAD, dtype=np.float32) % OT).reshape(1, OPAD), (K, 1)
    ).copy()
    return kcol, dpcol, jrow


def kernel(x, offsets, weight, bias, _trace=False, _trace_kwargs=None):
    x = np.asarray(x, dtype=np.float32)
    offsets = np.asarray(offsets, dtype=np.float32)
    weight = np.asarray(weight, dtype=np.float32)
    bias = np.asarray(bias, dtype=np.float32)

    if "nc" not in _prog_cache:
        _prog_cache["nc"] = _build_program()
    nc = _prog_cache["nc"]

    w_t = np.ascontiguousarray(
        np.transpose(weight, (1, 2, 0)).astype(ml_dtypes.bfloat16)
    )  # [c, k, d]
    bias2 = np.ascontiguousarray(bias.reshape(1, COUT).astype(ml_dtypes.bfloat16))
    kcol, dpcol, jrow = _consts()

    in_maps = []
    for core in range(8):
        b, half = core // 2, core % 2
        o_off = half * HALF
        xs = np.zeros((CIN, XW), dtype=ml_dtypes.bfloat16)
        xw = min(L - o_off, XW)
        xs[:, :xw] = x[b][:, o_off : o_off + xw].astype(ml_dtypes.bfloat16)
        offsT = np.zeros((K, OPAD), dtype=np.float32)
        ow = min(OUT_LEN - o_off, OPAD)
        offsT[:, :ow] = offsets[b, 0, o_off : o_off + ow, :].T
        in_maps.append(
            {
                "xs": xs, "wt": w_t, "offsT": offsT, "bias2": bias2,
                "jrow": jrow, "kcol": kcol, "dpcol": dpcol,
            }
        )

    if _trace:
        _install_axon_ntff_hook()
    try:
        res = run_bass_kernel_spmd(
            nc, in_maps, core_ids=list(range(8)),
            trace=_trace, **(_trace_kwargs or {}),
        )
    except Exception:
        # transient runtime faults have been observed; one retry
        res = run_bass_kernel_spmd(
            nc, in_maps, core_ids=list(range(8)),
            trace=_trace, **(_trace_kwargs or {}),
        )

    out = np.empty((B, COUT, OUT_LEN), dtype=np.float32)
    for core in range(8):
        b, half = core // 2, core % 2
        o_off = half * HALF
        out[b, :, o_off : o_off + HALF] = (
            res.results[core]["out"][:HALF, :].astype(np.float32).T
        )
    if _trace:
        _prog_cache["last_exec_time_ns"] = res.exec_time_ns
    return out


# revision 58
# speedup vs baseline: 1.0121x; 1.0121x over previous
"""DeformConv1d Trainium2 Bass kernel.

Problem: x[4,512,4096] f32, offsets[4,1,4090,7] f32, weight[512,512,7], bias[512]
  T[b,o,k]   = clamp(o + k + offsets[b,0,o,k], o, o+6)
  samp[b,c,o,k] = linear-interp of x[b,c,:] at T
  out[b,d,o] = sum_{c,k} samp[b,c,o,k] * weight[d,c,k] + bias[d]

Key identity: the clamp keeps every sample inside [o, o+6], so output o only
touches taps p in [o, o+7], and the interpolation weight of tap p is exactly
relu(1 - |p - T|).  With an o-tile of 121 the tap band is exactly 128 wide:

  out[o,d] = sum_{k, dp<128} S_k[dp, o] * Y[o0+dp, k, d]
    S_k[dp,o] = relu(1 - |cplus_k[o] - dp|),  cplus_k[o] = clamp(k+off,0,6)+j
    Y[p,k,d]  = sum_c x[c, p] * weight[d, c, k]

Both stages are dense bf16 matmuls on the PE array (f32 PSUM accumulate) and
run back-to-back at the 2-col/cycle bf16 stream rate (216 ns per N=512
matmul, ~86% tensor-engine occupancy).  cplus = clamp(k+off,0,6)+j is
computed once on-chip in f32, staged to DRAM as per-tile contiguous rows,
and expanded across all 128 partitions by a 0-stride DMA on the otherwise
idle software DGE queue (deep prefetch via an 8-buffer pool keeps it off the
critical path; tile 0's broadcast rides the hardware queue, wait-hinted
behind the weight stream to avoid head-of-line blocking it); ACT Abs with
per-partition bias -dp plus ACT relu turn it into the bf16 tap weights.
Bias is folded into the k=3 Y eviction (interp weights sum to 1 per k, so
adding bias[d] to one k's Y adds exactly bias[d] to the output) - no bias
matmul.  The output is stored bf16 (error budget allows it) to halve output
DMA traffic; the host upcasts.  PSUM evictions are split across
vector/scalar so no engine queue head-of-line-blocks stage-2.  The bulk x
stream is gated behind early output DMAs via scheduler wait hints so the
16 DMA engines (which round-robin all rung doorbells) give the head
bandwidth to the weights; the last tile's stage-2 runs in two d-halves so
the final eviction + store overlap its own matmuls.
Sharding: 8 cores = 4 batches x 2 halves of out_len (data parallel, no comm).
"""

import os
import sys

import ml_dtypes
import numpy as np

for _p in ("/opt/trn_rl_repo", os.path.expanduser("~/.axon_site/_ro/trn_rl_repo")):
    if os.path.isdir(_p) and _p not in sys.path:
        sys.path.insert(0, _p)

import concourse.mybir as mybir
import concourse.tile as tile
from concourse import bacc
from concourse.bass_utils import run_bass_kernel_spmd

B, CIN, COUT, L, K = 4, 512, 512, 4096, 7
OUT_LEN = 4090
HALF = 2045          # out positions per core (2 halves per batch)
OT = 121             # o-tile size -> tap band = OT + 7 = 128
TILES = 17           # 17 * 121 = 2057 >= 2045
OPAD = TILES * OT    # 2057 padded out positions per core
XW = (TILES - 1) * OT + 128  # 2064: rightmost x column any tile reads
P = 128
NCK = CIN // P       # 4 c-chunks
F32 = mybir.dt.float32
BF16 = mybir.dt.bfloat16

_prog_cache = {}


def _build_program():
    nc = bacc.Bacc("TRN2", target_bir_lowering=False, debug=False)

    xs_d = nc.dram_tensor("xs", [CIN, XW], BF16, kind="ExternalInput")
    wt_d = nc.dram_tensor("wt", [CIN, K, COUT], BF16, kind="ExternalInput")
    offsT_d = nc.dram_tensor("offsT", [K, OPAD], F32, kind="ExternalInput")
    bias_d = nc.dram_tensor("bias2", [1, COUT], BF16, kind="ExternalInput")
    jrow_d = nc.dram_tensor("jrow", [K, OPAD], F32, kind="ExternalInput")
    kcol_d = nc.dram_tensor("kcol", [K, 1], F32, kind="ExternalInput")
    dpcol_d = nc.dram_tensor("dpcol", [P, 1], F32, kind="ExternalInput")
    out_d = nc.dram_tensor("out", [OPAD, COUT], BF16, kind="ExternalOutput")

    with tile.TileContext(nc) as tc:
        with (
            tc.tile_pool(name="const", bufs=1) as cpool,
            tc.tile_pool(name="cdram", bufs=1, space="DRAM") as dpool,
            tc.tile_pool(name="cbt", bufs=8) as cbpool,
            tc.tile_pool(name="stiles", bufs=4) as stpool,
            tc.tile_pool(name="ytiles", bufs=3) as ypool,
            tc.tile_pool(name="otiles", bufs=3) as opool,
            tc.tile_pool(name="psy", bufs=7, space="PSUM") as psy,
            tc.tile_pool(name="pso", bufs=1, space="PSUM") as pso,
        ):
            # ---- small constants / offsets first (ahead of bulk x/W DMA) ----
            # the offsets -> cexp -> relayout -> cb(0) chain gates tile-0's
            # stage-2, so it leads the software queue; the 128-packet bias
            # broadcast rides the hardware queue instead (tiny, not urgent)
            offsT = cpool.tile([K, OPAD], F32)
            nc.gpsimd.dma_start(offsT[:], offsT_d[:])
            jrow = cpool.tile([K, OPAD], F32)
            nc.gpsimd.dma_start(jrow[:], jrow_d[:])
            kcol = cpool.tile([K, 1], F32)
            nc.gpsimd.dma_start(kcol[:], kcol_d[:])
            dpcol = cpool.tile([P, 1], F32)
            nc.gpsimd.dma_start(dpcol[:], dpcol_d[:])
            bias_bc = cpool.tile([P, COUT], BF16)
            with tc.tile_wait_until(0.006):
                nc.sync.dma_start(bias_bc[:], bias_d[:].partition_broadcast(P))

            # cplus[k, o] = clamp(k + off[k, o], 0, 6) + (o % OT), staged to
            # DRAM so a 0-stride DMA can expand it across partitions per tile
            cexp = cpool.tile([K, OPAD], F32)
            nc.vector.tensor_scalar(
                cexp[:], offsT[:], kcol[:], 0.0,
                mybir.AluOpType.add, mybir.AluOpType.max,
            )
            nc.vector.scalar_tensor_tensor(
                cexp[:], cexp[:], 6.0, jrow[:],
                mybir.AluOpType.min, mybir.AluOpType.add,
            )
            # relayout to per-tile contiguous rows [t, koff+j] in one shot:
            # k<4 at koff=121k, k>=4 at 512+121(k-4)
            c_dram2 = dpool.tile([TILES, 1024], F32)
            nc.gpsimd.dma_start(
                c_dram2[:, 0 : 4 * OT].rearrange("t (k j) -> k t j", k=4),
                cexp[0:4, :].rearrange("k (t j) -> k t j", t=TILES),
            )
            nc.gpsimd.dma_start(
                c_dram2[:, 512 : 512 + 3 * OT].rearrange("t (k j) -> k t j", k=3),
                cexp[4:7, :].rearrange("k (t j) -> k t j", t=TILES),
            )

            # ---- bulk inputs, in tile-0 consumption order ----
            xs = cpool.tile([P, NCK, XW], BF16)
            wt = cpool.tile([P, NCK, K, COUT], BF16)
            xs_src = xs_d[:].rearrange("(ci p) t -> p ci t", p=P)
            wt_src = wt_d[:].rearrange("(ci p) k d -> p ci k d", p=P)
            # head-critical: x band + all weights for tiles 0-2, ungated
            nc.sync.dma_start(xs[:, :, 0:130], xs_src[:, :, 0:130])
            for k in range(K):
                nc.sync.dma_start(wt[:, :, k, :], wt_src[:, :, k, :])
            with tc.tile_wait_until(0.0045):
                nc.sync.dma_start(xs[:, :, 130:391], xs_src[:, :, 130:391])
            # remaining x gated behind early output DMAs via scheduler wait
            # hints, so its packets don't round-robin-steal head bandwidth
            # from the weight stream (DMA engines serve all rung doorbells)
            for lo, hi, w in ((391, 775, 0.009), (775, 1420, 0.015),
                              (1420, XW, 0.028)):
                with tc.tile_wait_until(w):
                    nc.sync.dma_start(xs[:, :, lo:hi], xs_src[:, :, lo:hi])

            reg = slice(0, 512 + 3 * OT)
            for t in range(TILES):
                o0 = t * OT

                # ---- Y[dp, k, d] for band p in [o0, o0+128) ----
                y_sb = ypool.tile([P, K, COUT], BF16, tag="y_sb")
                for k in range(K):
                    yp = psy.tile([P, COUT], F32, tag="yp")
                    for ci in range(NCK):
                        nc.tensor.matmul(
                            yp[:],
                            xs[:, ci, o0 : o0 + P],
                            wt[:, ci, k, :],
                            start=(ci == 0), stop=(ci == NCK - 1),
                        )
                    if k == 3:
                        # interp weights sum to 1 per k: bias rides on k=3
                        nc.vector.tensor_tensor(
                            y_sb[:, k, :], yp[:], bias_bc[:],
                            mybir.AluOpType.add,
                        )
                    elif k in (0, 1, 2, 4):
                        nc.vector.tensor_copy(y_sb[:, k, :], yp[:])
                    else:
                        nc.scalar.copy(y_sb[:, k, :], yp[:])

                # ---- S_k[dp, o] = relu(1 - |cplus_k - dp|) ----
                # 0-stride DMA expands this tile's cplus row across all 128
                # partitions (4KB contiguous per partition), then ACT Abs with
                # per-partition bias -dp + ACT relu produce the bf16 weights.
                cb = cbpool.tile([P, 512 + 3 * OT], F32, tag="cb")
                s_sb = stpool.tile([P, 1024], BF16, tag="s_sb")
                # broadcasts ride the otherwise idle software DMA queue with
                # deep prefetch via cbpool, keeping the hardware queue free
                # for the bulk x/w input stream at the head; tile 0's rides
                # the fast hardware queue so s_sb(0) beats tile-0 stage-2
                if t == 0:
                    with tc.tile_wait_until(0.0045):
                        nc.sync.dma_start(
                            cb[:],
                            c_dram2[t : t + 1, 0 : 512 + 3 * OT].partition_broadcast(P),
                        )
                else:
                    nc.gpsimd.dma_start(
                        cb[:],
                        c_dram2[t : t + 1, 0 : 512 + 3 * OT].partition_broadcast(P),
                    )
                nc.scalar.activation(
                    cb[:], cb[:],
                    mybir.ActivationFunctionType.Abs,
                    bias=dpcol[:],
                )
                nc.scalar.activation(
                    s_sb[:, reg], cb[:],
                    mybir.ActivationFunctionType.Relu,
                    bias=1.0, scale=-1.0,
                )

                # ---- out[o, d] = sum_k S_k^T Y_k (+bias via k=3) ----
                # last tile runs in two d-halves so the first half's
                # eviction + store overlap the second half's matmuls,
                # shortening the serial tail after the final matmul
                op = pso.tile([P, COUT], F32, tag="op")
                o_sb = opool.tile([P, COUT], BF16, tag="o_sb")
                halves = ((0, 256), (256, COUT)) if t == TILES - 1 else ((0, COUT),)
                for dlo, dhi in halves:
                    for k in range(K):
                        koff = k * OT if k < 4 else 512 + (k - 4) * OT
                        nc.tensor.matmul(
                            op[:OT, dlo:dhi],
                            s_sb[:, koff : koff + OT],
                            y_sb[:, k, dlo:dhi],
                            start=(k == 0), stop=(k == K - 1),
                        )
                    # evict [o, d] bf16 and store; host upcasts + transposes
                    if t % 2 == 0:
                        nc.scalar.copy(o_sb[:OT, dlo:dhi], op[:OT, dlo:dhi])
                    else:
                        nc.vector.tensor_copy(o_sb[:OT, dlo:dhi], op[:OT, dlo:dhi])
                    nc.sync.dma_start(
                        out_d[o0 : o0 + OT, dlo:dhi], o_sb[:OT, dlo:dhi]
                    )

    nc.compile()
    return nc


def _install_axon_ntff_hook():
    """Provide antenv.axon_hooks (absent on this image) so that
    run_bass_kernel_spmd(trace=True) can capture NTFF profiles via the
    axon .so's C ABI.  Mirrors trn_agent_boot.trn_boot."""
    import contextlib
    import ctypes
    import types

    try:
        from antenv.axon_hooks import set_axon_ntff_profile_hook  # noqa: F401
        return
    except ImportError:
        pass

    so_path = "/opt/axon/libaxon_pjrt.so"
    if not os.path.exists(so_path):
        return
    lib = ctypes.CDLL(so_path)
    if not hasattr(lib, "axon_start_nrt_profile"):
        return
    lib.axon_start_nrt_profile.argtypes = [
        ctypes.POINTER(ctypes.c_int64), ctypes.c_size_t,
    ]
    lib.axon_start_nrt_profile.restype = ctypes.c_int64
    lib.axon_stop_nrt_profile.argtypes = [ctypes.c_char_p]
    lib.axon_stop_nrt_profile.restype = ctypes.c_int64

    @contextlib.contextmanager
    def _hook(output_dir, device_ids):
        import jax

        jax.devices()
        if device_ids:
            ids = (ctypes.c_int64 * len(device_ids))(*device_ids)
            rc = lib.axon_start_nrt_profile(ids, len(device_ids))
        else:
            rc = lib.axon_start_nrt_profile(None, 0)
        if rc != 0:
            raise RuntimeError(f"axon_start_nrt_profile rc={rc}")
        try:
            yield
        finally:
            n = lib.axon_stop_nrt_profile(str(output_dir).encode())
            print(f"ntff profile: {n} file(s) written to {output_dir}")

    box = {"h": _hook}
    mod = types.ModuleType("antenv.axon_hooks")
    mod.get_axon_ntff_profile_hook = lambda: box["h"]
    mod.set_axon_ntff_profile_hook = lambda h: box.__setitem__("h", h)
    import antenv

    sys.modules["antenv.axon_hooks"] = mod
    antenv.axon_hooks = mod

    # zero-egress env: skip the artifact upload in the trace path
    from concourse import bass_utils as _bu

    _bu.upload_artifacts = lambda d: f"local:{d}"


def _consts():
    kcol = np.arange(K, dtype=np.float32).reshape(K, 1).copy()
    # ACT Abs computes |x + bias|; bias = -dp gives |cplus - dp|
    dpcol = -np.arange(P, dtype=np.float32).reshape(P, 1)
    jrow = np.tile(
        (np.arange(OPAD, dtype=np.float32) % OT).reshape(1, OPAD), (K, 1)
    ).copy()
    return kcol, dpcol, jrow


def kernel(x, offsets, weight, bias, _trace=False, _trace_kwargs=None):
    x = np.asarray(x, dtype=np.float32)
    offsets = np.asarray(offsets, dtype=np.float32)
    weight = np.asarray(weight, dtype=np.float32)
    bias = np.asarray(bias, dtype=np.float32)

    if "nc" not in _prog_cache:
        _prog_cache["nc"] = _build_program()
    nc = _prog_cache["nc"]

    w_t = np.ascontiguousarray(
        np.transpose(weight, (1, 2, 0)).astype(ml_dtypes.bfloat16)
    )  # [c, k, d]
    bias2 = np.ascontiguousarray(bias.reshape(1, COUT).astype(ml_dtypes.bfloat16))
    kcol, dpcol, jrow = _consts()

    in_maps = []
    for core in range(8):
        b, half = core // 2, core % 2
        o_off = half * HALF
        xs = np.zeros((CIN, XW), dtype=ml_dtypes.bfloat16)
        xw = min(L - o_off, XW)
        xs[:, :xw] = x[b][:, o_off : o_off + xw].astype(ml_dtypes.bfloat16)
        offsT = np.zeros((K, OPAD), dtype=np.float32)
        ow = min(OUT_LEN - o_off, OPAD)
        offsT[:, :ow] = offsets[b, 0, o_off : o_off + ow, :].T
        in_maps.append(
            {
                "xs": xs, "wt": w_t, "offsT": offsT, "bias2": bias2,
                "jrow": jrow, "kcol": kcol, "dpcol": dpcol,
            }
        )

    if _trace:
        _install_axon_ntff_hook()
    try:
        res = run_bass_kernel_spmd(
            nc, in_maps, core_ids=list(range(8)),
            trace=_trace, **(_trace_kwargs or {}),
        )
    except Exception:
        # transient runtime faults have been observed; one retry
        res = run_bass_kernel_spmd(
            nc, in_maps, core_ids=list(range(8)),
            trace=_trace, **(_trace_kwargs or {}),
        )

    out = np.empty((B, COUT, OUT_LEN), dtype=np.float32)
    for core in range(8):
        b, half = core // 2, core % 2
        o_off = half * HALF
        out[b, :, o_off : o_off + HALF] = (
            res.results[core]["out"][:HALF, :].astype(np.float32).T
        )
    if _trace:
        _prog_cache["last_exec_time_ns"] = res.exec_time_ns
    return out
